# revision 21
# baseline (speedup 1.0000x reference)
"""Trainium2 Bass kernel for nn_Connector_77738908057780 (dense_mlp).

Computation (see reference):
  x   = image_features                      [B, N, H]    bf16
  f1  = mean(hidden[0:13],  axis=0)         [B, N, H]
  f2  = mean(hidden[13:26], axis=0)         [B, N, H]
  cat = concat([x, f1, f2], -1)             [B, N, 3H]
  h   = gelu(cat @ W1.T + b1)               W1 = nf4_dequant(codes1, scales1) [H, 3H]
  fg  = h @ W2.T + b2                       W2 = nf4_dequant(codes2, scales2) [H, H]
  out = w * LN(fg) + (1-w) * LN(x),         w = sigmoid(alpha)

Sharding: data-parallel over batch B=8 -> one batch element per NeuronCore.

Per-core plan:
  - the 26-layer sums are produced by SWDGE accumulate-DMA (CCE add) straight
    into SBUF: zero engine cost, pure DMA.  1/13 is folded into W1 host-side.
  - x/s1/s2 are transposed to feature-major via xbar DMA-transpose (SBUF->SBUF)
    to form cat^T tiles; GEMM1 runs weights-stationary producing h^T in PSUM;
    GELU(+b1 per-partition bias) on ACT gives g^T which directly feeds GEMM2
    as the stationary operand, producing fg back in token-major layout.
  - LayerNorm stats via DVE bn_stats/bn_aggr; rsqrt via DVE reciprocal + ACT
    sqrt (batched per supertile to limit activation-table switches); the
    normalize + sigmoid-gate combine is 4 fused scalar_tensor_tensor ops.

NF4 dequant of the (small, replicated) weights is host-side weight prep; the
bf16 weights are less DMA traffic than the int32 codes.
"""

import os
import sys

import numpy as np
import ml_dtypes

for _p in ("/opt/trn_rl_repo", "/root/.axon_site/_ro/trn_rl_repo"):
    if os.path.isdir(_p) and _p not in sys.path:
        sys.path.insert(0, _p)

import concourse.bass as bass
import concourse.mybir as mybir
import concourse.tile as tile
from concourse import bacc
from concourse import bass_utils

BF16 = mybir.dt.bfloat16
F32 = mybir.dt.float32
AF = mybir.ActivationFunctionType
ALU = mybir.AluOpType

NP_BF16 = ml_dtypes.bfloat16

P = 128
H = 1152
H3 = 3456
NT = 729          # tokens per core (N); B=8 cores
L = 26
KO1 = H3 // P     # 27 k-tiles for GEMM1
KO2 = H // P      # 9 k-tiles for GEMM2
MO = H // P       # 9 output-feature tiles
EPS = 1e-5
NCHUNK = 3        # fg free-dim chunks of 384
CH = H // NCHUNK  # 384

# Supertiles of exactly 256 tokens; the last overlaps the previous by 39
# tokens (473..511 computed twice, identical values stored twice) so that
# every DMA/compute tile is a full 128-partition tile (729 is not a
# multiple of 128; partial-partition tiles hit HW-hostile DMA paths).
SUPERTILES = [0, 256, 473]
TSUP = 256        # tokens per supertile
NSUB = 2          # 128-token subtiles per supertile

NF4_CODEBOOK = np.array([
    -1.0, -0.6961928009986877, -0.5250730514526367, -0.39491748809814453,
    -0.28444138169288635, -0.18477343022823334, -0.09105003625154495, 0.0,
    0.07958029955625534, 0.16093020141124725, 0.24611230194568634,
    0.33791524171829224, 0.4407098591327667, 0.5626170039176941,
    0.7229568362236023, 1.0], dtype=np.float32)

BLOCK = 64


def _dequant_nf4(codes, scales):
    """Match reference: codebook lookup * per-64-block absmax, cast bf16."""
    out_f, in_f = codes.shape
    w = NF4_CODEBOOK[codes].reshape(out_f, in_f // BLOCK, BLOCK)
    w = w * scales[:, :, None].astype(np.float32)
    return w.reshape(out_f, in_f)  # float32 (caller casts)


def _build_program(act=AF.Gelu):
    nc = bacc.Bacc(
        "TRN2",
        target_bir_lowering=False,
        debug=False,
        num_devices=1,
    )
    x_d = nc.dram_tensor("x", (NT, H), BF16, kind="ExternalInput").ap()
    hid_d = nc.dram_tensor("hid", (L, NT, H), BF16, kind="ExternalInput").ap()
    w1t_d = nc.dram_tensor("w1t", (H3, H), BF16, kind="ExternalInput").ap()
    w2t_d = nc.dram_tensor("w2t", (H, H), BF16, kind="ExternalInput").ap()
    b1s_d = nc.dram_tensor("b1s", (P, MO), F32, kind="ExternalInput").ap()
    b2b_d = nc.dram_tensor("b2b", (P, H), F32, kind="ExternalInput").ap()
    g1b_d = nc.dram_tensor("g1b", (P, H), BF16, kind="ExternalInput").ap()
    g2b_d = nc.dram_tensor("g2b", (P, H), BF16, kind="ExternalInput").ap()
    bcb_d = nc.dram_tensor("bcb", (P, H), BF16, kind="ExternalInput").ap()
    out_d = nc.dram_tensor("out", (NT, H), BF16, kind="ExternalOutput").ap()

    with tile.TileContext(nc) as tc:
        _program(nc, tc, x_d, hid_d, w1t_d, w2t_d, b1s_d, b2b_d,
                 g1b_d, g2b_d, bcb_d, out_d, act)

    nc.compile()
    return nc


def _program(nc, tc, x_d, hid_d, w1t_d, w2t_d, b1s_d, b2b_d, g1b_d, g2b_d,
             bcb_d, out_d, act=AF.Gelu):
    with (
        tc.tile_pool(name="consts", bufs=1) as cpool,
        tc.tile_pool(name="hl", bufs=10) as hpool,
        tc.tile_pool(name="acc", bufs=2) as apool,
        tc.tile_pool(name="cat", bufs=1) as catpool,
        tc.tile_pool(name="gt", bufs=1) as gpool,
        tc.tile_pool(name="xn", bufs=2) as xpool,
        tc.tile_pool(name="fg", bufs=2) as fgpool,
        tc.tile_pool(name="outp", bufs=2) as opool,
        tc.tile_pool(name="stats", bufs=2) as spool,
        tc.tile_pool(name="tmp", bufs=1) as tpool,
        tc.tile_pool(name="ps", bufs=4, space="PSUM") as pspool,
    ):
        # ---- constants (loaded once) ----
        # All weights/consts go on the scalar HWDGE queue so the sync queue
        # starts streaming `hidden` immediately; w1t is loaded in 3 k-chunks
        # so GEMM1's early k-tiles unblock as soon as their chunk lands.
        w1t_sb = cpool.tile([P, KO1, H], BF16)
        w1t_r = w1t_d.rearrange("(ko p) n -> p ko n", p=P)
        for c0 in range(0, KO1, 9):
            nc.scalar.dma_start(w1t_sb[:, c0:c0 + 9, :], w1t_r[:, c0:c0 + 9, :])
        w2t_sb = cpool.tile([P, KO2, H], BF16)
        nc.scalar.dma_start(w2t_sb, w2t_d.rearrange("(ko p) n -> p ko n", p=P))
        b1_sb = cpool.tile([P, MO], F32)
        nc.scalar.dma_start(b1_sb, b1s_d)
        b2b_sb = cpool.tile([P, H], F32)
        nc.scalar.dma_start(b2b_sb, b2b_d)
        g1b_sb = cpool.tile([P, H], BF16)
        nc.scalar.dma_start(g1b_sb, g1b_d)
        g2b_sb = cpool.tile([P, H], BF16)
        nc.scalar.dma_start(g2b_sb, g2b_d)
        bcb_sb = cpool.tile([P, H], BF16)
        nc.scalar.dma_start(bcb_sb, bcb_d)

        for st_idx, t0 in enumerate(SUPERTILES):
            # ---- x (token-major, also used by LN1) ----
            x_nat = xpool.tile([P, NSUB, H], BF16, tag="xnat")
            nc.scalar.dma_start(
                x_nat,
                x_d[t0:t0 + TSUP, :].rearrange("(s p) f -> p s f", p=P),
            )

            catT = catpool.tile([P, NSUB, KO1, P], BF16, tag="catT")
            # x^T chunk of cat^T can transpose as soon as x lands
            for tt in range(NSUB):
                nc.scalar.dma_start_transpose(catT[:, tt, 0:MO, :],
                                              x_nat[:, tt, :])

            # ---- 26-layer sums: plain HWDGE loads, adds split DVE/GPSIMD --
            # (accumulate-DMA RMW costs ~2x DMA-engine time; DVE has slack)
            def accum_half(l_start, tag):
                layers = []
                for i in range(13):
                    lt = hpool.tile([P, NSUB, H], BF16, name=f"hl{tag}{i}",
                                    tag="hl")
                    eng = nc.sync if (i % 2 == 0) else nc.scalar
                    eng.dma_start(
                        lt,
                        hid_d[l_start + i, t0:t0 + TSUP, :].rearrange(
                            "(s p) f -> p s f", p=P))
                    layers.append(lt)
                # DVE sums layers 0..8, GPSIMD sums 9..12, DVE combines
                acc = apool.tile([P, NSUB, H], BF16, name=f"s{tag}", tag=tag)
                nc.vector.tensor_add(acc, layers[0], layers[1])
                for i in range(2, 9):
                    nc.vector.tensor_add(acc, acc, layers[i])
                accb = apool.tile([P, NSUB, H], BF16, name=f"sb{tag}",
                                  tag=tag + "b", bufs=1)
                nc.gpsimd.tensor_add(accb, layers[9], layers[10])
                nc.gpsimd.tensor_add(accb, accb, layers[11])
                nc.gpsimd.tensor_add(accb, accb, layers[12])
                nc.vector.tensor_add(acc, acc, accb)
                return acc

            s1 = accum_half(0, "s1")
            s2 = accum_half(13, "s2")

            # ---- f1^T/f2^T chunks of cat^T via xbar transpose (SBUF->SBUF)
            # catT[pf, tt, ko, t] = cat[token(tt,t), ko*128+pf]
            for tt in range(NSUB):
                nc.sync.dma_start_transpose(catT[:, tt, MO:2 * MO, :],
                                            s1[:, tt, :])
                nc.sync.dma_start_transpose(catT[:, tt, 2 * MO:3 * MO, :],
                                            s2[:, tt, :])

            # ---- GEMM1: h^T[oh, t] = W1'^T-stationary matmuls; GELU -> g^T
            gT = gpool.tile([P, MO, TSUP], BF16, tag="gT")
            for mm in range(MO):
                ps1 = pspool.tile([P, TSUP], F32, tag="ps")
                for kk in range(KO1):
                    nc.tensor.matmul(
                        ps1.rearrange("p (a b) -> p a b", a=NSUB),
                        lhsT=w1t_sb[:, kk, mm * P:(mm + 1) * P],
                        rhs=catT[:, :, kk, :],
                        start=(kk == 0),
                        stop=(kk == KO1 - 1),
                    )
                nc.scalar.activation(gT[:, mm, :], ps1, act,
                                     bias=b1_sb[:, mm:mm + 1])

            # ---- per-supertile packed rsqrt input: var+eps for 2 LNs x 3 sub
            rpack = spool.tile([P, 2 * NSUB], F32, tag="rpack")
            agg = spool.tile([P, NSUB, 4], F32, tag="agg")

            fgs = []
            for tt in range(NSUB):
                # ---- GEMM2: fg[t, oh2] = g^T-stationary matmuls (+b2)
                fg = fgpool.tile([P, H], BF16, tag="fg")
                fgs.append(fg)
                for nn in range(NCHUNK):
                    ps2 = pspool.tile([P, CH], F32, tag="ps")
                    for kk in range(KO2):
                        nc.tensor.matmul(
                            ps2,
                            lhsT=gT[:, kk, tt * P:(tt + 1) * P],
                            rhs=w2t_sb[:, kk, nn * CH:(nn + 1) * CH],
                            start=(kk == 0),
                            stop=(kk == KO2 - 1),
                        )
                    nc.vector.tensor_tensor(
                        fg[:, nn * CH:(nn + 1) * CH], ps2,
                        b2b_sb[:, nn * CH:(nn + 1) * CH], ALU.add)

                # ---- LN stats (mean/var over H) for x and fg ----
                bnx = spool.tile([P, 3, 6], F32, tag="bnx")
                for c in range(NCHUNK):
                    nc.vector.bn_stats(bnx[:, c, :],
                                       x_nat[:, tt, c * CH:(c + 1) * CH])
                nc.vector.bn_aggr(agg[:, tt, 0:2], bnx)
                bnf = spool.tile([P, 3, 6], F32, tag="bnf")
                for c in range(NCHUNK):
                    nc.vector.bn_stats(bnf[:, c, :],
                                       fg[:, c * CH:(c + 1) * CH])
                nc.vector.bn_aggr(agg[:, tt, 2:4], bnf)
                nc.vector.tensor_scalar_add(rpack[:, 2 * tt:2 * tt + 1],
                                            agg[:, tt, 1:2], EPS)
                nc.vector.tensor_scalar_add(rpack[:, 2 * tt + 1:2 * tt + 2],
                                            agg[:, tt, 3:4], EPS)

            # ---- rsqrt batched: one reciprocal (DVE) + one sqrt (ACT) ----
            ig = spool.tile([P, 2 * NSUB], F32, tag="ig")
            nc.vector.reciprocal(ig, rpack)
            nc.scalar.activation(ig, ig, AF.Sqrt)

            # ---- normalize + sigmoid gate, store ----
            for tt in range(NSUB):
                fg = fgs[tt]
                tmp1 = tpool.tile([P, H], BF16, tag="tmp1")
                tmp2 = tpool.tile([P, H], BF16, tag="tmp2")
                # tmp1 = (x - mu1) * G1;  G1 = (1-w)*ln1_g  (broadcast)
                nc.vector.scalar_tensor_tensor(
                    tmp1, x_nat[:, tt, :], agg[:, tt, 0:1], g1b_sb,
                    ALU.subtract, ALU.mult)
                # tmp2 = (fg - mu2) * G2;  G2 = w*ln2_g
                nc.vector.scalar_tensor_tensor(
                    tmp2, fg, agg[:, tt, 2:3], g2b_sb,
                    ALU.subtract, ALU.mult)
                # tmp1 = tmp1 * ig1 + Bc;  Bc = w*ln2_b + (1-w)*ln1_b
                nc.vector.scalar_tensor_tensor(
                    tmp1, tmp1, ig[:, 2 * tt:2 * tt + 1], bcb_sb,
                    ALU.mult, ALU.add)
                # out = tmp2 * ig2 + tmp1
                out_t = opool.tile([P, H], BF16, tag="outt")
                nc.vector.scalar_tensor_tensor(
                    out_t, tmp2, ig[:, 2 * tt + 1:2 * tt + 2], tmp1,
                    ALU.mult, ALU.add)
                nc.sync.dma_start(
                    out_d[t0 + tt * P:t0 + (tt + 1) * P, :], out_t)


_NC_CACHE = {}


def _get_nc():
    if "nc" not in _NC_CACHE:
        _NC_CACHE["nc"] = _build_program()
    return _NC_CACHE["nc"]


def _host_prep(codes1, scales1, b1, codes2, scales2, b2,
               ln1_g, ln1_b, ln2_g, ln2_b, alpha):
    # W1 with 1/13 folded into the f1/f2 column blocks (mean -> sum)
    w1 = _dequant_nf4(codes1, scales1)
    # match reference rounding: dequant result is cast to bf16 first
    w1 = w1.astype(NP_BF16).astype(np.float32)
    w1[:, H:] *= np.float32(1.0 / 13.0)
    w1t = np.ascontiguousarray(w1.T).astype(NP_BF16)

    w2 = _dequant_nf4(codes2, scales2).astype(NP_BF16)
    w2t = np.ascontiguousarray(w2.astype(np.float32).T).astype(NP_BF16)

    b1s = np.ascontiguousarray(
        b1.astype(np.float32).reshape(MO, P).T)  # [P, MO]

    b2b = np.ascontiguousarray(
        np.broadcast_to(b2.astype(np.float32), (P, H)))

    a32 = alpha.astype(np.float32)
    w_gate = (1.0 / (1.0 + np.exp(-a32[0]))).astype(NP_BF16)
    one_minus = (NP_BF16(1.0) - w_gate)
    g1 = (one_minus.astype(np.float32) * ln1_g.astype(np.float32))
    g2 = (w_gate.astype(np.float32) * ln2_g.astype(np.float32))
    bc = (w_gate.astype(np.float32) * ln2_b.astype(np.float32)
          + one_minus.astype(np.float32) * ln1_b.astype(np.float32))
    g1b = np.ascontiguousarray(np.broadcast_to(g1.astype(NP_BF16), (P, H)))
    g2b = np.ascontiguousarray(np.broadcast_to(g2.astype(NP_BF16), (P, H)))
    bcb = np.ascontiguousarray(np.broadcast_to(bc.astype(NP_BF16), (P, H)))
    return w1t, w2t, b1s, b2b, g1b, g2b, bcb


def make_in_maps(image_features, hidden, codes1, scales1, b1, codes2, scales2,
                 b2, ln1_g, ln1_b, ln2_g, ln2_b, alpha):
    w1t, w2t, b1s, b2b, g1b, g2b, bcb = _host_prep(
        codes1, scales1, b1, codes2, scales2, b2,
        ln1_g, ln1_b, ln2_g, ln2_b, alpha)
    B = image_features.shape[0]
    in_maps = []
    for c in range(B):
        in_maps.append({
            "x": np.ascontiguousarray(image_features[c]).astype(NP_BF16, copy=False),
            "hid": np.ascontiguousarray(hidden[:, c]).astype(NP_BF16, copy=False),
            "w1t": w1t, "w2t": w2t, "b1s": b1s, "b2b": b2b,
            "g1b": g1b, "g2b": g2b, "bcb": bcb,
        })
    return in_maps


def kernel(image_features, hidden, codes1, scales1, b1, codes2, scales2, b2,
           ln1_g, ln1_b, ln2_g, ln2_b, alpha, _trace=False):
    B, N, Hin = image_features.shape
    assert (B, N, Hin) == (8, NT, H), (B, N, Hin)
    nc = _get_nc()
    in_maps = make_in_maps(image_features, hidden, codes1, scales1, b1,
                           codes2, scales2, b2, ln1_g, ln1_b, ln2_g, ln2_b,
                           alpha)
    res = bass_utils.run_bass_kernel_spmd(
        nc, in_maps, core_ids=list(range(8)), trace=_trace)
    out = np.stack([res.results[c]["out"] for c in range(8)])
    if _trace:
        kernel._last_results = res
    return out.astype(image_features.dtype, copy=False)


# revision 22
# speedup vs baseline: 1.1475x; 1.1475x over previous
"""Trainium2 Bass kernel for nn_Connector_77738908057780 (dense_mlp).

Computation (see reference):
  x   = image_features                      [B, N, H]    bf16
  f1  = mean(hidden[0:13],  axis=0)         [B, N, H]
  f2  = mean(hidden[13:26], axis=0)         [B, N, H]
  cat = concat([x, f1, f2], -1)             [B, N, 3H]
  h   = gelu(cat @ W1.T + b1)               W1 = nf4_dequant(codes1, scales1) [H, 3H]
  fg  = h @ W2.T + b2                       W2 = nf4_dequant(codes2, scales2) [H, H]
  out = w * LN(fg) + (1-w) * LN(x),         w = sigmoid(alpha)

Sharding: data-parallel over batch B=8 -> one batch element per NeuronCore.

Per-core plan:
  - the 26-layer sums are produced by SWDGE accumulate-DMA (CCE add) straight
    into SBUF: zero engine cost, pure DMA.  1/13 is folded into W1 host-side.
  - x/s1/s2 are transposed to feature-major via xbar DMA-transpose (SBUF->SBUF)
    to form cat^T tiles; GEMM1 runs weights-stationary producing h^T in PSUM;
    GELU(+b1 per-partition bias) on ACT gives g^T which directly feeds GEMM2
    as the stationary operand, producing fg back in token-major layout.
  - LayerNorm stats via DVE bn_stats/bn_aggr; rsqrt via DVE reciprocal + ACT
    sqrt (batched per supertile to limit activation-table switches); the
    normalize + sigmoid-gate combine is 4 fused scalar_tensor_tensor ops.

NF4 dequant of the (small, replicated) weights is host-side weight prep; the
bf16 weights are less DMA traffic than the int32 codes.
"""

import os
import sys

import numpy as np
import ml_dtypes

for _p in ("/opt/trn_rl_repo", "/root/.axon_site/_ro/trn_rl_repo"):
    if os.path.isdir(_p) and _p not in sys.path:
        sys.path.insert(0, _p)

import concourse.bass as bass
import concourse.mybir as mybir
import concourse.tile as tile
from concourse import bacc
from concourse import bass_utils

BF16 = mybir.dt.bfloat16
F32 = mybir.dt.float32
AF = mybir.ActivationFunctionType
ALU = mybir.AluOpType

NP_BF16 = ml_dtypes.bfloat16

P = 128
H = 1152
H3 = 3456
NT = 729          # tokens per core (N); B=8 cores
L = 26
KO1 = H3 // P     # 27 k-tiles for GEMM1
KO2 = H // P      # 9 k-tiles for GEMM2
MO = H // P       # 9 output-feature tiles
EPS = 1e-5
NCHUNK = 3        # fg free-dim chunks of 384
CH = H // NCHUNK  # 384

# Supertiles of exactly 256 tokens; the last overlaps the previous by 39
# tokens (473..511 computed twice, identical values stored twice) so that
# every DMA/compute tile is a full 128-partition tile (729 is not a
# multiple of 128; partial-partition tiles hit HW-hostile DMA paths).
SUPERTILES = [0, 256, 473]
TSUP = 256        # tokens per supertile
NSUB = 2          # 128-token subtiles per supertile

NF4_CODEBOOK = np.array([
    -1.0, -0.6961928009986877, -0.5250730514526367, -0.39491748809814453,
    -0.28444138169288635, -0.18477343022823334, -0.09105003625154495, 0.0,
    0.07958029955625534, 0.16093020141124725, 0.24611230194568634,
    0.33791524171829224, 0.4407098591327667, 0.5626170039176941,
    0.7229568362236023, 1.0], dtype=np.float32)

BLOCK = 64


def _dequant_nf4(codes, scales):
    """Match reference: codebook lookup * per-64-block absmax, cast bf16."""
    out_f, in_f = codes.shape
    w = NF4_CODEBOOK[codes].reshape(out_f, in_f // BLOCK, BLOCK)
    w = w * scales[:, :, None].astype(np.float32)
    return w.reshape(out_f, in_f)  # float32 (caller casts)


def _build_program(act=AF.Gelu):
    nc = bacc.Bacc(
        "TRN2",
        target_bir_lowering=False,
        debug=False,
        num_devices=1,
    )
    x_d = nc.dram_tensor("x", (NT, H), BF16, kind="ExternalInput").ap()
    hid_d = nc.dram_tensor("hid", (L, NT, H), BF16, kind="ExternalInput").ap()
    w1t_d = nc.dram_tensor("w1t", (H3, H), BF16, kind="ExternalInput").ap()
    w2t_d = nc.dram_tensor("w2t", (H, H), BF16, kind="ExternalInput").ap()
    b1s_d = nc.dram_tensor("b1s", (P, MO), F32, kind="ExternalInput").ap()
    b2b_d = nc.dram_tensor("b2b", (P, H), F32, kind="ExternalInput").ap()
    g1b_d = nc.dram_tensor("g1b", (P, H), BF16, kind="ExternalInput").ap()
    g2b_d = nc.dram_tensor("g2b", (P, H), BF16, kind="ExternalInput").ap()
    bcb_d = nc.dram_tensor("bcb", (P, H), BF16, kind="ExternalInput").ap()
    out_d = nc.dram_tensor("out", (NT, H), BF16, kind="ExternalOutput").ap()

    with tile.TileContext(nc) as tc:
        _program(nc, tc, x_d, hid_d, w1t_d, w2t_d, b1s_d, b2b_d,
                 g1b_d, g2b_d, bcb_d, out_d, act)

    nc.compile()
    return nc


def _program(nc, tc, x_d, hid_d, w1t_d, w2t_d, b1s_d, b2b_d, g1b_d, g2b_d,
             bcb_d, out_d, act=AF.Gelu):
    with (
        tc.tile_pool(name="consts", bufs=1) as cpool,
        tc.tile_pool(name="hl", bufs=10) as hpool,
        tc.tile_pool(name="acc", bufs=2) as apool,
        tc.tile_pool(name="cat", bufs=1) as catpool,
        tc.tile_pool(name="gt", bufs=1) as gpool,
        tc.tile_pool(name="xn", bufs=2) as xpool,
        tc.tile_pool(name="fg", bufs=2) as fgpool,
        tc.tile_pool(name="outp", bufs=2) as opool,
        tc.tile_pool(name="stats", bufs=2) as spool,
        tc.tile_pool(name="tmp", bufs=2) as tpool,
        tc.tile_pool(name="ps", bufs=4, space="PSUM") as pspool,
    ):
        # ---- constants (loaded once) ----
        # All weights/consts go on the scalar HWDGE queue so the sync queue
        # starts streaming `hidden` immediately; w1t is loaded in 3 k-chunks
        # so GEMM1's early k-tiles unblock as soon as their chunk lands.
        w1t_sb = cpool.tile([P, KO1, H], BF16)
        w1t_r = w1t_d.rearrange("(ko p) n -> p ko n", p=P)
        for c0 in range(0, KO1, 9):
            nc.scalar.dma_start(w1t_sb[:, c0:c0 + 9, :], w1t_r[:, c0:c0 + 9, :])
        w2t_sb = cpool.tile([P, KO2, H], BF16)
        nc.sync.dma_start(w2t_sb, w2t_d.rearrange("(ko p) n -> p ko n", p=P))
        b1_sb = cpool.tile([P, MO], F32)
        nc.sync.dma_start(b1_sb, b1s_d)
        b2b_sb = cpool.tile([P, H], F32)
        nc.sync.dma_start(b2b_sb, b2b_d)
        g1b_sb = cpool.tile([P, H], BF16)
        nc.sync.dma_start(g1b_sb, g1b_d)
        g2b_sb = cpool.tile([P, H], BF16)
        nc.sync.dma_start(g2b_sb, g2b_d)
        bcb_sb = cpool.tile([P, H], BF16)
        nc.sync.dma_start(bcb_sb, bcb_d)

        for st_idx, t0 in enumerate(SUPERTILES):
            # ---- x (token-major, also used by LN1) ----
            x_nat = xpool.tile([P, NSUB, H], BF16, tag="xnat")
            nc.scalar.dma_start(
                x_nat,
                x_d[t0:t0 + TSUP, :].rearrange("(s p) f -> p s f", p=P),
            )

            catT = catpool.tile([P, NSUB, KO1, P], BF16, tag="catT")
            # x^T chunk of cat^T can transpose as soon as x lands
            for tt in range(NSUB):
                nc.scalar.dma_start_transpose(catT[:, tt, 0:MO, :],
                                              x_nat[:, tt, :])

            # ---- 26-layer sums: plain HWDGE loads, adds split DVE/GPSIMD --
            # (accumulate-DMA RMW costs ~2x DMA-engine time; DVE has slack)
            def accum_half(l_start, tag):
                layers = []
                for i in range(13):
                    lt = hpool.tile([P, NSUB, H], BF16, name=f"hl{tag}{i}",
                                    tag="hl")
                    eng = nc.sync if (i % 2 == 0) else nc.scalar
                    eng.dma_start(
                        lt,
                        hid_d[l_start + i, t0:t0 + TSUP, :].rearrange(
                            "(s p) f -> p s f", p=P))
                    layers.append(lt)
                acc = apool.tile([P, NSUB, H], BF16, name=f"s{tag}", tag=tag)
                nc.vector.tensor_add(acc, layers[0], layers[1])
                for i in range(2, 13):
                    nc.vector.tensor_add(acc, acc, layers[i])
                return acc

            s1 = accum_half(0, "s1")
            s2 = accum_half(13, "s2")

            # ---- f1^T/f2^T chunks of cat^T via xbar transpose (SBUF->SBUF)
            # catT[pf, tt, ko, t] = cat[token(tt,t), ko*128+pf]
            for tt in range(NSUB):
                nc.sync.dma_start_transpose(catT[:, tt, MO:2 * MO, :],
                                            s1[:, tt, :])
                nc.sync.dma_start_transpose(catT[:, tt, 2 * MO:3 * MO, :],
                                            s2[:, tt, :])

            # ---- GEMM1: h^T[oh, t] = W1'^T-stationary matmuls; GELU -> g^T
            gT = gpool.tile([P, MO, TSUP], BF16, tag="gT")
            for mm in range(MO):
                ps1 = pspool.tile([P, TSUP], F32, tag="ps")
                for kk in range(KO1):
                    nc.tensor.matmul(
                        ps1.rearrange("p (a b) -> p a b", a=NSUB),
                        lhsT=w1t_sb[:, kk, mm * P:(mm + 1) * P],
                        rhs=catT[:, :, kk, :],
                        start=(kk == 0),
                        stop=(kk == KO1 - 1),
                    )
                nc.scalar.activation(gT[:, mm, :], ps1, act,
                                     bias=b1_sb[:, mm:mm + 1])

            # ---- per-supertile packed rsqrt input: var+eps for 2 LNs x 3 sub
            rpack = spool.tile([P, 2 * NSUB], F32, tag="rpack")
            agg = spool.tile([P, NSUB, 4], F32, tag="agg")

            fgs = []
            for tt in range(NSUB):
                # ---- GEMM2: fg[t, oh2] = g^T-stationary matmuls (+b2)
                fg = fgpool.tile([P, H], BF16, tag="fg")
                fgs.append(fg)
                for nn in range(NCHUNK):
                    ps2 = pspool.tile([P, CH], F32, tag="ps")
                    for kk in range(KO2):
                        nc.tensor.matmul(
                            ps2,
                            lhsT=gT[:, kk, tt * P:(tt + 1) * P],
                            rhs=w2t_sb[:, kk, nn * CH:(nn + 1) * CH],
                            start=(kk == 0),
                            stop=(kk == KO2 - 1),
                        )
                    nc.vector.tensor_tensor(
                        fg[:, nn * CH:(nn + 1) * CH], ps2,
                        b2b_sb[:, nn * CH:(nn + 1) * CH], ALU.add)

                # ---- LN stats (mean/var over H) for x and fg ----
                bnx = spool.tile([P, 3, 6], F32, tag="bnx")
                for c in range(NCHUNK):
                    nc.vector.bn_stats(bnx[:, c, :],
                                       x_nat[:, tt, c * CH:(c + 1) * CH])
                nc.vector.bn_aggr(agg[:, tt, 0:2], bnx)
                bnf = spool.tile([P, 3, 6], F32, tag="bnf")
                for c in range(NCHUNK):
                    nc.vector.bn_stats(bnf[:, c, :],
                                       fg[:, c * CH:(c + 1) * CH])
                nc.vector.bn_aggr(agg[:, tt, 2:4], bnf)
                nc.vector.tensor_scalar_add(rpack[:, 2 * tt:2 * tt + 1],
                                            agg[:, tt, 1:2], EPS)
                nc.vector.tensor_scalar_add(rpack[:, 2 * tt + 1:2 * tt + 2],
                                            agg[:, tt, 3:4], EPS)

            # ---- rsqrt batched: one reciprocal (DVE) + one sqrt (ACT) ----
            ig = spool.tile([P, 2 * NSUB], F32, tag="ig")
            nc.vector.reciprocal(ig, rpack)
            nc.scalar.activation(ig, ig, AF.Sqrt)

            # ---- normalize + sigmoid gate, store ----
            for tt in range(NSUB):
                fg = fgs[tt]
                tmp1 = tpool.tile([P, H], BF16, tag="tmp1")
                tmp2 = tpool.tile([P, H], BF16, tag="tmp2")
                # tmp1 = (x - mu1) * G1;  G1 = (1-w)*ln1_g  (broadcast)
                nc.vector.scalar_tensor_tensor(
                    tmp1, x_nat[:, tt, :], agg[:, tt, 0:1], g1b_sb,
                    ALU.subtract, ALU.mult)
                # tmp2 = (fg - mu2) * G2;  G2 = w*ln2_g
                nc.vector.scalar_tensor_tensor(
                    tmp2, fg, agg[:, tt, 2:3], g2b_sb,
                    ALU.subtract, ALU.mult)
                # tmp1 = tmp1 * ig1 + Bc;  Bc = w*ln2_b + (1-w)*ln1_b
                nc.vector.scalar_tensor_tensor(
                    tmp1, tmp1, ig[:, 2 * tt:2 * tt + 1], bcb_sb,
                    ALU.mult, ALU.add)
                # out = tmp2 * ig2 + tmp1
                out_t = opool.tile([P, H], BF16, tag="outt")
                nc.vector.scalar_tensor_tensor(
                    out_t, tmp2, ig[:, 2 * tt + 1:2 * tt + 2], tmp1,
                    ALU.mult, ALU.add)
                nc.sync.dma_start(
                    out_d[t0 + tt * P:t0 + (tt + 1) * P, :], out_t)


_NC_CACHE = {}


def _get_nc():
    if "nc" not in _NC_CACHE:
        _NC_CACHE["nc"] = _build_program()
    return _NC_CACHE["nc"]


def _host_prep(codes1, scales1, b1, codes2, scales2, b2,
               ln1_g, ln1_b, ln2_g, ln2_b, alpha):
    # W1 with 1/13 folded into the f1/f2 column blocks (mean -> sum)
    w1 = _dequant_nf4(codes1, scales1)
    # match reference rounding: dequant result is cast to bf16 first
    w1 = w1.astype(NP_BF16).astype(np.float32)
    w1[:, H:] *= np.float32(1.0 / 13.0)
    w1t = np.ascontiguousarray(w1.T).astype(NP_BF16)

    w2 = _dequant_nf4(codes2, scales2).astype(NP_BF16)
    w2t = np.ascontiguousarray(w2.astype(np.float32).T).astype(NP_BF16)

    b1s = np.ascontiguousarray(
        b1.astype(np.float32).reshape(MO, P).T)  # [P, MO]

    b2b = np.ascontiguousarray(
        np.broadcast_to(b2.astype(np.float32), (P, H)))

    a32 = alpha.astype(np.float32)
    w_gate = (1.0 / (1.0 + np.exp(-a32[0]))).astype(NP_BF16)
    one_minus = (NP_BF16(1.0) - w_gate)
    g1 = (one_minus.astype(np.float32) * ln1_g.astype(np.float32))
    g2 = (w_gate.astype(np.float32) * ln2_g.astype(np.float32))
    bc = (w_gate.astype(np.float32) * ln2_b.astype(np.float32)
          + one_minus.astype(np.float32) * ln1_b.astype(np.float32))
    g1b = np.ascontiguousarray(np.broadcast_to(g1.astype(NP_BF16), (P, H)))
    g2b = np.ascontiguousarray(np.broadcast_to(g2.astype(NP_BF16), (P, H)))
    bcb = np.ascontiguousarray(np.broadcast_to(bc.astype(NP_BF16), (P, H)))
    return w1t, w2t, b1s, b2b, g1b, g2b, bcb


def make_in_maps(image_features, hidden, codes1, scales1, b1, codes2, scales2,
                 b2, ln1_g, ln1_b, ln2_g, ln2_b, alpha):
    w1t, w2t, b1s, b2b, g1b, g2b, bcb = _host_prep(
        codes1, scales1, b1, codes2, scales2, b2,
        ln1_g, ln1_b, ln2_g, ln2_b, alpha)
    B = image_features.shape[0]
    in_maps = []
    for c in range(B):
        in_maps.append({
            "x": np.ascontiguousarray(image_features[c]).astype(NP_BF16, copy=False),
            "hid": np.ascontiguousarray(hidden[:, c]).astype(NP_BF16, copy=False),
            "w1t": w1t, "w2t": w2t, "b1s": b1s, "b2b": b2b,
            "g1b": g1b, "g2b": g2b, "bcb": bcb,
        })
    return in_maps


def kernel(image_features, hidden, codes1, scales1, b1, codes2, scales2, b2,
           ln1_g, ln1_b, ln2_g, ln2_b, alpha, _trace=False):
    B, N, Hin = image_features.shape
    assert (B, N, Hin) == (8, NT, H), (B, N, Hin)
    nc = _get_nc()
    in_maps = make_in_maps(image_features, hidden, codes1, scales1, b1,
                           codes2, scales2, b2, ln1_g, ln1_b, ln2_g, ln2_b,
                           alpha)
    res = bass_utils.run_bass_kernel_spmd(
        nc, in_maps, core_ids=list(range(8)), trace=_trace)
    out = np.stack([res.results[c]["out"] for c in range(8)])
    if _trace:
        kernel._last_results = res
    return out.astype(image_features.dtype, copy=False)


# revision 23
# speedup vs baseline: 1.1687x; 1.0185x over previous
"""Trainium2 Bass kernel for nn_Connector_77738908057780 (dense_mlp).

Computation (see reference):
  x   = image_features                      [B, N, H]    bf16
  f1  = mean(hidden[0:13],  axis=0)         [B, N, H]
  f2  = mean(hidden[13:26], axis=0)         [B, N, H]
  cat = concat([x, f1, f2], -1)             [B, N, 3H]
  h   = gelu(cat @ W1.T + b1)               W1 = nf4_dequant(codes1, scales1) [H, 3H]
  fg  = h @ W2.T + b2                       W2 = nf4_dequant(codes2, scales2) [H, H]
  out = w * LN(fg) + (1-w) * LN(x),         w = sigmoid(alpha)

Sharding: data-parallel over batch B=8 -> one batch element per NeuronCore.

Per-core plan:
  - the 26-layer sums are produced by SWDGE accumulate-DMA (CCE add) straight
    into SBUF: zero engine cost, pure DMA.  1/13 is folded into W1 host-side.
  - x/s1/s2 are transposed to feature-major via xbar DMA-transpose (SBUF->SBUF)
    to form cat^T tiles; GEMM1 runs weights-stationary producing h^T in PSUM;
    GELU(+b1 per-partition bias) on ACT gives g^T which directly feeds GEMM2
    as the stationary operand, producing fg back in token-major layout.
  - LayerNorm stats via DVE bn_stats/bn_aggr; rsqrt via DVE reciprocal + ACT
    sqrt (batched per supertile to limit activation-table switches); the
    normalize + sigmoid-gate combine is 4 fused scalar_tensor_tensor ops.

NF4 dequant of the (small, replicated) weights is host-side weight prep; the
bf16 weights are less DMA traffic than the int32 codes.
"""

import os
import sys

import numpy as np
import ml_dtypes

for _p in ("/opt/trn_rl_repo", "/root/.axon_site/_ro/trn_rl_repo"):
    if os.path.isdir(_p) and _p not in sys.path:
        sys.path.insert(0, _p)

import concourse.bass as bass
import concourse.mybir as mybir
import concourse.tile as tile
from concourse import bacc
from concourse import bass_utils

BF16 = mybir.dt.bfloat16
F32 = mybir.dt.float32
AF = mybir.ActivationFunctionType
ALU = mybir.AluOpType

NP_BF16 = ml_dtypes.bfloat16

P = 128
H = 1152
H3 = 3456
NT = 729          # tokens per core (N); B=8 cores
L = 26
KO1 = H3 // P     # 27 k-tiles for GEMM1
KO2 = H // P      # 9 k-tiles for GEMM2
MO = H // P       # 9 output-feature tiles
EPS = 1e-5
NCHUNK = 3        # fg free-dim chunks of 384
CH = H // NCHUNK  # 384

# Supertiles of exactly 256 tokens; the last overlaps the previous by 39
# tokens (473..511 computed twice, identical values stored twice) so that
# every DMA/compute tile is a full 128-partition tile (729 is not a
# multiple of 128; partial-partition tiles hit HW-hostile DMA paths).
SUPERTILES = [0, 256, 473]
TSUP = 256        # tokens per supertile
NSUB = 2          # 128-token subtiles per supertile

NF4_CODEBOOK = np.array([
    -1.0, -0.6961928009986877, -0.5250730514526367, -0.39491748809814453,
    -0.28444138169288635, -0.18477343022823334, -0.09105003625154495, 0.0,
    0.07958029955625534, 0.16093020141124725, 0.24611230194568634,
    0.33791524171829224, 0.4407098591327667, 0.5626170039176941,
    0.7229568362236023, 1.0], dtype=np.float32)

BLOCK = 64


def _dequant_nf4(codes, scales):
    """Match reference: codebook lookup * per-64-block absmax, cast bf16."""
    out_f, in_f = codes.shape
    w = NF4_CODEBOOK[codes].reshape(out_f, in_f // BLOCK, BLOCK)
    w = w * scales[:, :, None].astype(np.float32)
    return w.reshape(out_f, in_f)  # float32 (caller casts)


def _build_program(act=AF.Gelu):
    nc = bacc.Bacc(
        "TRN2",
        target_bir_lowering=False,
        debug=False,
        num_devices=1,
    )
    x_d = nc.dram_tensor("x", (NT, H), BF16, kind="ExternalInput").ap()
    hid_d = nc.dram_tensor("hid", (L, NT, H), BF16, kind="ExternalInput").ap()
    w1t_d = nc.dram_tensor("w1t", (H3, H), BF16, kind="ExternalInput").ap()
    w2t_d = nc.dram_tensor("w2t", (H, H), BF16, kind="ExternalInput").ap()
    b1s_d = nc.dram_tensor("b1s", (P, MO), F32, kind="ExternalInput").ap()
    b2b_d = nc.dram_tensor("b2b", (P, H), F32, kind="ExternalInput").ap()
    g1b_d = nc.dram_tensor("g1b", (P, H), BF16, kind="ExternalInput").ap()
    g2b_d = nc.dram_tensor("g2b", (P, H), BF16, kind="ExternalInput").ap()
    bcb_d = nc.dram_tensor("bcb", (P, H), BF16, kind="ExternalInput").ap()
    out_d = nc.dram_tensor("out", (NT, H), BF16, kind="ExternalOutput").ap()

    with tile.TileContext(nc) as tc:
        _program(nc, tc, x_d, hid_d, w1t_d, w2t_d, b1s_d, b2b_d,
                 g1b_d, g2b_d, bcb_d, out_d, act)

    nc.compile()
    return nc


def _program(nc, tc, x_d, hid_d, w1t_d, w2t_d, b1s_d, b2b_d, g1b_d, g2b_d,
             bcb_d, out_d, act=AF.Gelu):
    with (
        tc.tile_pool(name="consts", bufs=1) as cpool,
        tc.tile_pool(name="hl", bufs=10) as hpool,
        tc.tile_pool(name="acc", bufs=2) as apool,
        tc.tile_pool(name="cat", bufs=1) as catpool,
        tc.tile_pool(name="gt", bufs=1) as gpool,
        tc.tile_pool(name="xn", bufs=2) as xpool,
        tc.tile_pool(name="fg", bufs=2) as fgpool,
        tc.tile_pool(name="outp", bufs=2) as opool,
        tc.tile_pool(name="stats", bufs=2) as spool,
        tc.tile_pool(name="tmp", bufs=2) as tpool,
        tc.tile_pool(name="ps", bufs=4, space="PSUM") as pspool,
    ):
        # ---- constants (loaded once) ----
        # All weights/consts go on the scalar HWDGE queue so the sync queue
        # starts streaming `hidden` immediately; w1t is loaded in 3 k-chunks
        # so GEMM1's early k-tiles unblock as soon as their chunk lands.
        w1t_sb = cpool.tile([P, KO1, H], BF16)
        w1t_r = w1t_d.rearrange("(ko p) n -> p ko n", p=P)
        for c0 in range(0, KO1, 9):
            nc.scalar.dma_start(w1t_sb[:, c0:c0 + 9, :], w1t_r[:, c0:c0 + 9, :])
        w2t_sb = cpool.tile([P, KO2, H], BF16)
        nc.sync.dma_start(w2t_sb, w2t_d.rearrange("(ko p) n -> p ko n", p=P))
        b1_sb = cpool.tile([P, MO], F32)
        nc.sync.dma_start(b1_sb, b1s_d)
        b2b_sb = cpool.tile([P, H], F32)
        nc.sync.dma_start(b2b_sb, b2b_d)
        g1b_sb = cpool.tile([P, H], BF16)
        nc.sync.dma_start(g1b_sb, g1b_d)
        g2b_sb = cpool.tile([P, H], BF16)
        nc.sync.dma_start(g2b_sb, g2b_d)
        bcb_sb = cpool.tile([P, H], BF16)
        nc.sync.dma_start(bcb_sb, bcb_d)

        for st_idx, t0 in enumerate(SUPERTILES):
            # ---- x (token-major, also used by LN1) ----
            x_nat = xpool.tile([P, NSUB, H], BF16, tag="xnat")
            nc.scalar.dma_start(
                x_nat,
                x_d[t0:t0 + TSUP, :].rearrange("(s p) f -> p s f", p=P),
            )

            catT = catpool.tile([P, NSUB, KO1, P], BF16, tag="catT")
            # x^T chunk of cat^T can transpose as soon as x lands
            for tt in range(NSUB):
                nc.scalar.dma_start_transpose(catT[:, tt, 0:MO, :],
                                              x_nat[:, tt, :])

            # ---- 26-layer sums: plain HWDGE loads, adds split DVE/GPSIMD --
            # (accumulate-DMA RMW costs ~2x DMA-engine time; DVE has slack)
            def accum_half(l_start, tag):
                layers = []
                for i in range(13):
                    lt = hpool.tile([P, NSUB, H], BF16, name=f"hl{tag}{i}",
                                    tag="hl")
                    eng = nc.sync if (i % 2 == 0) else nc.scalar
                    eng.dma_start(
                        lt,
                        hid_d[l_start + i, t0:t0 + TSUP, :].rearrange(
                            "(s p) f -> p s f", p=P))
                    layers.append(lt)
                acc = apool.tile([P, NSUB, H], BF16, name=f"s{tag}", tag=tag)
                nc.vector.tensor_add(acc, layers[0], layers[1])
                for i in range(2, 13):
                    nc.vector.tensor_add(acc, acc, layers[i])
                return acc

            s1 = accum_half(0, "s1")
            s2 = accum_half(13, "s2")

            # ---- f1^T/f2^T chunks of cat^T via xbar transpose (SBUF->SBUF)
            # catT[pf, tt, ko, t] = cat[token(tt,t), ko*128+pf]
            for tt in range(NSUB):
                nc.scalar.dma_start_transpose(catT[:, tt, MO:2 * MO, :],
                                              s1[:, tt, :])
                nc.scalar.dma_start_transpose(catT[:, tt, 2 * MO:3 * MO, :],
                                              s2[:, tt, :])

            # ---- GEMM1: h^T[oh, t] = W1'^T-stationary matmuls; GELU -> g^T
            gT = gpool.tile([P, MO, TSUP], BF16, tag="gT")
            for mm in range(MO):
                ps1 = pspool.tile([P, TSUP], F32, tag="ps")
                for kk in range(KO1):
                    nc.tensor.matmul(
                        ps1.rearrange("p (a b) -> p a b", a=NSUB),
                        lhsT=w1t_sb[:, kk, mm * P:(mm + 1) * P],
                        rhs=catT[:, :, kk, :],
                        start=(kk == 0),
                        stop=(kk == KO1 - 1),
                    )
                nc.scalar.activation(gT[:, mm, :], ps1, act,
                                     bias=b1_sb[:, mm:mm + 1])

            # ---- per-supertile packed rsqrt input: var+eps for 2 LNs x 3 sub
            rpack = spool.tile([P, 2 * NSUB], F32, tag="rpack")
            agg = spool.tile([P, NSUB, 4], F32, tag="agg")

            fgs = []
            for tt in range(NSUB):
                # ---- GEMM2: fg[t, oh2] = g^T-stationary matmuls (+b2)
                fg = fgpool.tile([P, H], BF16, tag="fg")
                fgs.append(fg)
                for nn in range(NCHUNK):
                    ps2 = pspool.tile([P, CH], F32, tag="ps")
                    for kk in range(KO2):
                        nc.tensor.matmul(
                            ps2,
                            lhsT=gT[:, kk, tt * P:(tt + 1) * P],
                            rhs=w2t_sb[:, kk, nn * CH:(nn + 1) * CH],
                            start=(kk == 0),
                            stop=(kk == KO2 - 1),
                        )
                    nc.vector.tensor_tensor(
                        fg[:, nn * CH:(nn + 1) * CH], ps2,
                        b2b_sb[:, nn * CH:(nn + 1) * CH], ALU.add)

                # ---- LN stats (mean/var over H) for x and fg ----
                bnx = spool.tile([P, 3, 6], F32, tag="bnx")
                for c in range(NCHUNK):
                    nc.vector.bn_stats(bnx[:, c, :],
                                       x_nat[:, tt, c * CH:(c + 1) * CH])
                nc.vector.bn_aggr(agg[:, tt, 0:2], bnx)
                bnf = spool.tile([P, 3, 6], F32, tag="bnf")
                for c in range(NCHUNK):
                    nc.vector.bn_stats(bnf[:, c, :],
                                       fg[:, c * CH:(c + 1) * CH])
                nc.vector.bn_aggr(agg[:, tt, 2:4], bnf)
                nc.vector.tensor_scalar_add(rpack[:, 2 * tt:2 * tt + 1],
                                            agg[:, tt, 1:2], EPS)
                nc.vector.tensor_scalar_add(rpack[:, 2 * tt + 1:2 * tt + 2],
                                            agg[:, tt, 3:4], EPS)

            # ---- rsqrt batched: one reciprocal (DVE) + one sqrt (ACT) ----
            ig = spool.tile([P, 2 * NSUB], F32, tag="ig")
            nc.vector.reciprocal(ig, rpack)
            nc.scalar.activation(ig, ig, AF.Sqrt)

            # ---- normalize + sigmoid gate, store ----
            for tt in range(NSUB):
                fg = fgs[tt]
                tmp1 = tpool.tile([P, H], BF16, tag="tmp1")
                tmp2 = tpool.tile([P, H], BF16, tag="tmp2")
                # tmp1 = (x - mu1) * G1;  G1 = (1-w)*ln1_g  (broadcast)
                nc.vector.scalar_tensor_tensor(
                    tmp1, x_nat[:, tt, :], agg[:, tt, 0:1], g1b_sb,
                    ALU.subtract, ALU.mult)
                # tmp2 = (fg - mu2) * G2;  G2 = w*ln2_g
                nc.vector.scalar_tensor_tensor(
                    tmp2, fg, agg[:, tt, 2:3], g2b_sb,
                    ALU.subtract, ALU.mult)
                # tmp1 = tmp1 * ig1 + Bc;  Bc = w*ln2_b + (1-w)*ln1_b
                nc.vector.scalar_tensor_tensor(
                    tmp1, tmp1, ig[:, 2 * tt:2 * tt + 1], bcb_sb,
                    ALU.mult, ALU.add)
                # out = tmp2 * ig2 + tmp1
                out_t = opool.tile([P, H], BF16, tag="outt")
                nc.vector.scalar_tensor_tensor(
                    out_t, tmp2, ig[:, 2 * tt + 1:2 * tt + 2], tmp1,
                    ALU.mult, ALU.add)
                nc.scalar.dma_start(
                    out_d[t0 + tt * P:t0 + (tt + 1) * P, :], out_t)


_NC_CACHE = {}


def _get_nc():
    if "nc" not in _NC_CACHE:
        _NC_CACHE["nc"] = _build_program()
    return _NC_CACHE["nc"]


def _host_prep(codes1, scales1, b1, codes2, scales2, b2,
               ln1_g, ln1_b, ln2_g, ln2_b, alpha):
    # W1 with 1/13 folded into the f1/f2 column blocks (mean -> sum)
    w1 = _dequant_nf4(codes1, scales1)
    # match reference rounding: dequant result is cast to bf16 first
    w1 = w1.astype(NP_BF16).astype(np.float32)
    w1[:, H:] *= np.float32(1.0 / 13.0)
    w1t = np.ascontiguousarray(w1.T).astype(NP_BF16)

    w2 = _dequant_nf4(codes2, scales2).astype(NP_BF16)
    w2t = np.ascontiguousarray(w2.astype(np.float32).T).astype(NP_BF16)

    b1s = np.ascontiguousarray(
        b1.astype(np.float32).reshape(MO, P).T)  # [P, MO]

    b2b = np.ascontiguousarray(
        np.broadcast_to(b2.astype(np.float32), (P, H)))

    a32 = alpha.astype(np.float32)
    w_gate = (1.0 / (1.0 + np.exp(-a32[0]))).astype(NP_BF16)
    one_minus = (NP_BF16(1.0) - w_gate)
    g1 = (one_minus.astype(np.float32) * ln1_g.astype(np.float32))
    g2 = (w_gate.astype(np.float32) * ln2_g.astype(np.float32))
    bc = (w_gate.astype(np.float32) * ln2_b.astype(np.float32)
          + one_minus.astype(np.float32) * ln1_b.astype(np.float32))
    g1b = np.ascontiguousarray(np.broadcast_to(g1.astype(NP_BF16), (P, H)))
    g2b = np.ascontiguousarray(np.broadcast_to(g2.astype(NP_BF16), (P, H)))
    bcb = np.ascontiguousarray(np.broadcast_to(bc.astype(NP_BF16), (P, H)))
    return w1t, w2t, b1s, b2b, g1b, g2b, bcb


def make_in_maps(image_features, hidden, codes1, scales1, b1, codes2, scales2,
                 b2, ln1_g, ln1_b, ln2_g, ln2_b, alpha):
    w1t, w2t, b1s, b2b, g1b, g2b, bcb = _host_prep(
        codes1, scales1, b1, codes2, scales2, b2,
        ln1_g, ln1_b, ln2_g, ln2_b, alpha)
    B = image_features.shape[0]
    in_maps = []
    for c in range(B):
        in_maps.append({
            "x": np.ascontiguousarray(image_features[c]).astype(NP_BF16, copy=False),
            "hid": np.ascontiguousarray(hidden[:, c]).astype(NP_BF16, copy=False),
            "w1t": w1t, "w2t": w2t, "b1s": b1s, "b2b": b2b,
            "g1b": g1b, "g2b": g2b, "bcb": bcb,
        })
    return in_maps


def kernel(image_features, hidden, codes1, scales1, b1, codes2, scales2, b2,
           ln1_g, ln1_b, ln2_g, ln2_b, alpha, _trace=False):
    B, N, Hin = image_features.shape
    assert (B, N, Hin) == (8, NT, H), (B, N, Hin)
    nc = _get_nc()
    in_maps = make_in_maps(image_features, hidden, codes1, scales1, b1,
                           codes2, scales2, b2, ln1_g, ln1_b, ln2_g, ln2_b,
                           alpha)
    res = bass_utils.run_bass_kernel_spmd(
        nc, in_maps, core_ids=list(range(8)), trace=_trace)
    out = np.stack([res.results[c]["out"] for c in range(8)])
    if _trace:
        kernel._last_results = res
    return out.astype(image_features.dtype, copy=False)
